# revision 5
# baseline (speedup 1.0000x reference)
"""Trainium2 Bass kernel for nn_Attention_335007449334.

Strategy: tensor-parallel over heads across 8 NeuronCores.
  - Each core owns 4 query heads + 1 kv head: wq/wk/wv column-sharded.
  - x is transposed on-device (PE transpose of a D-shard, then AllGather)
    because the TensorE contracts along the partition axis.
  - Attention is computed fully transposed (scoresT [k, q]) so no
    probs-transpose is needed: softmax sums come from ones-matmuls, the
    causal mask is applied multiplicatively post-exp (exp(s+m)=exp(s)exp(m)),
    and normalization is folded into the PSUM->SBUF evacuation using
    row-broadcast reciprocal tiles.
  - RoPE works on an even/odd-interleaved head-dim permutation (baked into
    the host-side weight column order) so it becomes 6 aligned DVE ops.
  - attnT shards are AllGathered; wo is column-sharded, each core computes
    out^T[:, 512r:512r+512]; host concatenates + transposes.
All matmuls run as float32r (TF32-like, ~1e-4 rel err, full PE rate at N>=512).
"""

import sys
import numpy as np

sys.path.insert(0, "/opt/trn_rl_repo")

import concourse.bass as bass  # noqa: E402
import concourse.tile as tile  # noqa: E402
from concourse import bacc, mybir  # noqa: E402
from concourse.bass_utils import run_bass_kernel_spmd  # noqa: E402
from concourse.masks import make_identity  # noqa: E402

B, S, D = 4, 512, 4096
H, HK, HD = 32, 8, 128
NCORES = 8
HL = H // NCORES  # 4 local q-heads per core
A_LEN = 64
SCALE = 1.0 / float(np.sqrt(HD))

F32 = mybir.dt.float32
F32R = mybir.dt.float32r

_cache = {}
last_result = None


def _host_prep(inputs):
    x = np.ascontiguousarray(np.asarray(inputs["x"], np.float32).reshape(B * S, D))
    adapter = np.ascontiguousarray(
        np.asarray(inputs["adapter"], np.float32).reshape(B * A_LEN, D)
    )
    mask = np.asarray(inputs["mask"], np.float32)[0, 0]
    cos = np.asarray(inputs["freqs_cos"], np.float32)
    sin = np.asarray(inputs["freqs_sin"], np.float32)
    wq = np.asarray(inputs["wq"], np.float32)
    wk = np.asarray(inputs["wk"], np.float32)
    wv = np.asarray(inputs["wv"], np.float32)
    wo = np.asarray(inputs["wo"], np.float32)
    gate = np.asarray(inputs["gate"], np.float32)[0, :, 0, 0]
    tg = np.tanh(gate).astype(np.float32)

    canonical = np.where(
        np.tril(np.ones((S, S), dtype=bool)), np.float32(0.0), np.float32(-1e9)
    ).astype(np.float32)
    causal = bool(np.array_equal(mask, canonical))
    adapter_skip = bool(np.all(tg == 0.0))

    cosT = np.ascontiguousarray(cos.T)  # [64, S]
    sinT = np.ascontiguousarray(sin.T)
    C2 = np.ascontiguousarray(np.concatenate([cosT, cosT], axis=0))
    S2 = np.ascontiguousarray(np.concatenate([sinT, sinT], axis=0))
    S2a = np.ascontiguousarray(np.concatenate([-sinT, sinT], axis=0))
    S2b = np.ascontiguousarray(np.concatenate([sinT, -sinT], axis=0))

    if causal:
        em = np.ascontiguousarray(np.exp(mask[0:128, 0:128].T).astype(np.float32))
    else:
        em = np.ascontiguousarray(np.exp(mask.T).astype(np.float32))  # [k, q]

    ev = np.arange(0, HD, 2)
    od = np.arange(1, HD, 2)

    in_maps = []
    for r in range(NCORES):
        heads = [4 * r + i for i in range(HL)]
        cols = []
        for p in range(HL // 2):
            h0, h1 = heads[2 * p], heads[2 * p + 1]
            cols.append(np.concatenate([h0 * HD + ev, h1 * HD + ev]))
            cols.append(np.concatenate([h0 * HD + od, h1 * HD + od]))
        wq_r = np.ascontiguousarray(wq[:, np.concatenate(cols)])
        ka_cols = np.concatenate([r * HD + ev, r * HD + od])
        wk_r = np.ascontiguousarray(wk[:, ka_cols])
        wv_r = np.ascontiguousarray(wv[:, r * HD : (r + 1) * HD])
        wo_r = np.ascontiguousarray(wo[:, 512 * r : 512 * (r + 1)])
        x_sl = np.ascontiguousarray(x[:, 512 * r : 512 * (r + 1)])
        m = dict(
            x_sl=x_sl, adapter=adapter, wq_r=wq_r, wk_r=wk_r, wv_r=wv_r,
            wo_r=wo_r, C2=C2, S2=S2, S2a=S2a, S2b=S2b, em=em,
        )
        if not adapter_skip:
            m["tg4"] = np.ascontiguousarray(tg[4 * r : 4 * r + 4].reshape(4, 1))
        in_maps.append(m)
    return in_maps, causal, adapter_skip


def _build(causal, adapter_skip):
    nc = bacc.Bacc(trn_type="TRN2", num_devices=NCORES)

    x_sl = nc.dram_tensor("x_sl", [B * S, 512], F32, kind="ExternalInput")
    ad = nc.dram_tensor("adapter", [B * A_LEN, D], F32, kind="ExternalInput")
    wq_r = nc.dram_tensor("wq_r", [D, 512], F32R, kind="ExternalInput")
    wk_r = nc.dram_tensor("wk_r", [D, 128], F32R, kind="ExternalInput")
    wv_r = nc.dram_tensor("wv_r", [D, 128], F32R, kind="ExternalInput")
    wo_r = nc.dram_tensor("wo_r", [D, 512], F32R, kind="ExternalInput")
    c2_d = nc.dram_tensor("C2", [128, S], F32, kind="ExternalInput")
    s2_d = nc.dram_tensor("S2", [128, S], F32, kind="ExternalInput")
    s2a_d = nc.dram_tensor("S2a", [128, S], F32, kind="ExternalInput")
    s2b_d = nc.dram_tensor("S2b", [128, S], F32, kind="ExternalInput")
    em_shape = [128, 128] if causal else [S, S]
    em_d = nc.dram_tensor("em", em_shape, F32, kind="ExternalInput")
    if not adapter_skip:
        tg_d = nc.dram_tensor("tg4", [HL, 1], F32, kind="ExternalInput")
    out_r = nc.dram_tensor("out_r", [512, B * S], F32, kind="ExternalOutput")

    rg = [list(range(NCORES))]

    with tile.TileContext(nc) as tc:
        with (
            tc.tile_pool(name="const", bufs=1) as constp,
            tc.tile_pool(name="wres", bufs=1) as wres,
            tc.tile_pool(name="dram", bufs=1, space="DRAM") as dram,
        ):
            # ---- constants ----
            ident = constp.tile([128, 128], F32)
            make_identity(nc, ident[:])
            ones_f = constp.tile([128, 1], F32)
            nc.vector.memset(ones_f[:], 1.0)
            ones_r = constp.tile([128, 1], F32R)
            nc.vector.tensor_copy(ones_r[:], ones_f[:])
            c2 = constp.tile([128, S], F32)
            nc.sync.dma_start(c2[:], c2_d[:])
            s2 = constp.tile([128, S], F32)
            nc.sync.dma_start(s2[:], s2_d[:])
            s2a = constp.tile([128, S], F32)
            nc.sync.dma_start(s2a[:], s2a_d[:])
            s2b = constp.tile([128, S], F32)
            nc.sync.dma_start(s2b[:], s2b_d[:])
            em_sb = constp.tile(em_shape if causal else [128, 4, S], F32)
            if causal:
                nc.sync.dma_start(em_sb[:], em_d[:])
            else:
                nc.sync.dma_start(
                    em_sb[:], em_d[:].rearrange("(kc p) q -> p kc q", p=128)
                )
            if not adapter_skip:
                tg4 = constp.tile([HL, 1], F32)
                nc.sync.dma_start(tg4[:], tg_d[:])

            # ---- resident weights ----
            wqw = wres.tile([128, 32, 512], F32R)
            nc.sync.dma_start(wqw[:], wq_r[:].rearrange("(kt p) c -> p kt c", p=128))
            kaw = wres.tile([128, 32, 128], F32R)
            nc.sync.dma_start(kaw[:], wk_r[:].rearrange("(kt p) c -> p kt c", p=128))
            wvw = wres.tile([128, 32, 128], F32R)
            nc.sync.dma_start(wvw[:], wv_r[:].rearrange("(kt p) c -> p kt c", p=128))

            # ---- x shard transpose + AllGather ----
            xt_ag_in = dram.tile([512, B * S], F32R)
            xt_full = dram.tile([D, B * S], F32R, addr_space="Shared")
            with (
                tc.tile_pool(name="xtp", bufs=1) as xtp,
                tc.tile_pool(name="xtps", bufs=2, space="PSUM") as xtps,
            ):
                x_sb = xtp.tile([128, 16, 512], F32)
                nc.sync.dma_start(
                    x_sb[:], x_sl[:].rearrange("(tt p) c -> p tt c", p=128)
                )
                xts = xtp.tile([128, 4, B * S], F32R)
                for i in range(4):
                    for g in range(4):
                        pst = xtps.tile([128, 512], F32)
                        for j in range(4):
                            nc.tensor.transpose(
                                pst[:, 128 * j : 128 * (j + 1)],
                                x_sb[:, 4 * g + j, 128 * i : 128 * (i + 1)],
                                ident[:],
                            )
                        nc.scalar.copy(xts[:, i, 512 * g : 512 * (g + 1)], pst[:])
                nc.sync.dma_start(
                    xt_ag_in[:].rearrange("(i p) t -> p i t", p=128), xts[:]
                )
            nc.gpsimd.collective_compute(
                "AllGather", mybir.AluOpType.bypass, replica_groups=rg,
                ins=[xt_ag_in[:]], outs=[xt_full[:]],
            )

            # ---- adapter transpose + projections ----
            with (
                tc.tile_pool(name="adp", bufs=1) as adp,
                tc.tile_pool(name="adps", bufs=2, space="PSUM") as adps,
            ):
                ad_sb = adp.tile([128, 2, D], F32)
                nc.sync.dma_start(
                    ad_sb[:], ad[:].rearrange("(tt p) c -> p tt c", p=128)
                )
                adt = adp.tile([128, 32, B * A_LEN], F32R)
                for kt in range(32):
                    psa = adps.tile([128, 256], F32)
                    for j in range(2):
                        nc.tensor.transpose(
                            psa[:, 128 * j : 128 * (j + 1)],
                            ad_sb[:, j, 128 * kt : 128 * (kt + 1)],
                            ident[:],
                        )
                    nc.scalar.copy(adt[:, kt, :], psa[:])
                # aKT / aVT projections
                akt = wres.tile([128, B, A_LEN], F32R)
                aktb = wres.tile([128, B, A_LEN], F32R)
                avt = adp.tile([128, B * A_LEN], F32)
                pk = adps.tile([128, 256], F32)
                for kt in range(32):
                    nc.tensor.matmul(
                        pk[:], kaw[:, kt, :], adt[:, kt, :],
                        start=(kt == 0), stop=(kt == 31),
                    )
                nc.scalar.copy(
                    akt[:].rearrange("p b a -> p (b a)"), pk[:]
                )
                pv = adps.tile([128, 256], F32)
                for kt in range(32):
                    nc.tensor.matmul(
                        pv[:], wvw[:, kt, :], adt[:, kt, :],
                        start=(kt == 0), stop=(kt == 31),
                    )
                nc.scalar.copy(avt[:], pv[:])
                # aKTB = swapped halves of aKT
                nc.sync.dma_start(
                    aktb[0:64, :, :].bitcast(F32), akt[64:128, :, :].bitcast(F32)
                )
                nc.sync.dma_start(
                    aktb[64:128, :, :].bitcast(F32), akt[0:64, :, :].bitcast(F32)
                )
                # aV token-major per batch
                av_sb = wres.tile([64, B, 128], F32R)
                for b in range(B):
                    pav = adps.tile([64, 128], F32)
                    nc.tensor.transpose(
                        pav[:], avt[:, 64 * b : 64 * (b + 1)], ident[:]
                    )
                    nc.scalar.copy(av_sb[:, b, :], pav[:])

            # ---- per-batch QKV + attention ----
            at_in = []
            at_full = []
            for b in range(B):
                at_in.append(dram.tile([512, S], F32R, name=f"at_in{b}"))
                at_full.append(
                    dram.tile([D, S], F32R, addr_space="Shared", name=f"at_full{b}")
                )

            for b in range(B):
              with tc.tile_pool(name=f"bat{b}", bufs=1) as bat:
                with (
                    tc.tile_pool(name=f"xts{b}", bufs=3) as xstream,
                    tc.tile_pool(name=f"qkvps{b}", bufs=1, space="PSUM") as qkvps,
                    tc.tile_pool(name=f"vtps{b}", bufs=1, space="PSUM") as vtps,
                ):
                    q_ps = [
                        qkvps.tile([128, 512], F32, name=f"qps{m}") for m in range(4)
                    ]
                    k_ps = qkvps.tile([128, 512], F32)
                    v_ps = qkvps.tile([128, 512], F32)
                    for kt in range(32):
                        xt_t = xstream.tile([128, 512], F32R, tag="xt")
                        nc.sync.dma_start(
                            xt_t[:],
                            xt_full[128 * kt : 128 * (kt + 1), 512 * b : 512 * (b + 1)],
                        )
                        st, sp = (kt == 0), (kt == 31)
                        for m in range(4):
                            nc.tensor.matmul(
                                q_ps[m][:], wqw[:, kt, 128 * m : 128 * (m + 1)],
                                xt_t[:], start=st, stop=sp,
                            )
                        nc.tensor.matmul(k_ps[:], kaw[:, kt, :], xt_t[:], start=st, stop=sp)
                        nc.tensor.matmul(v_ps[:], wvw[:, kt, :], xt_t[:], start=st, stop=sp)

                    # RoPE on Q pair-blocks: rqA = QA*C2 - QB*S2 ; rqB = QA*S2 + QB*C2
                    rqa, rqb = [], []
                    for p in range(2):
                        qa, qb = q_ps[2 * p], q_ps[2 * p + 1]
                        t1 = bat.tile([128, S], F32, tag=f"t1_{p}")
                        t2 = bat.tile([128, S], F32, tag=f"t2_{p}")
                        ra = bat.tile([128, S], F32R, name=f"rqa{p}")
                        rb = bat.tile([128, S], F32R, name=f"rqb{p}")
                        nc.vector.tensor_mul(t1[:], qa[:], c2[:])
                        nc.vector.tensor_mul(t2[:], qb[:], s2[:])
                        nc.vector.tensor_sub(ra[:], t1[:], t2[:])
                        nc.vector.tensor_mul(t1[:], qa[:], s2[:])
                        nc.vector.tensor_mul(t2[:], qb[:], c2[:])
                        nc.vector.tensor_add(rb[:], t1[:], t2[:])
                        rqa.append(ra)
                        rqb.append(rb)
                    # K: ka/kb swap-duplicate, then rope
                    ka_f = bat.tile([128, S], F32)
                    nc.scalar.copy(ka_f[:], k_ps[:])
                    kb_f = bat.tile([128, S], F32)
                    nc.sync.dma_start(kb_f[0:64, :], ka_f[64:128, :])
                    nc.sync.dma_start(kb_f[64:128, :], ka_f[0:64, :])
                    t1 = bat.tile([128, S], F32, tag="kt1")
                    t2 = bat.tile([128, S], F32, tag="kt2")
                    rka = bat.tile([128, S], F32R)
                    rkb = bat.tile([128, S], F32R)
                    nc.vector.tensor_mul(t1[:], ka_f[:], c2[:])
                    nc.vector.tensor_mul(t2[:], kb_f[:], s2a[:])
                    nc.vector.tensor_add(rka[:], t1[:], t2[:])
                    nc.vector.tensor_mul(t1[:], kb_f[:], c2[:])
                    nc.vector.tensor_mul(t2[:], ka_f[:], s2b[:])
                    nc.vector.tensor_add(rkb[:], t1[:], t2[:])
                    # V: token-major
                    vt_f = bat.tile([128, S], F32)
                    nc.scalar.copy(vt_f[:], v_ps[:])
                    v_sb = bat.tile([128, 4, 128], F32R)
                    for kc in range(4):
                        pv2 = vtps.tile([128, 128], F32)
                        nc.tensor.transpose(
                            pv2[:], vt_f[:, 128 * kc : 128 * (kc + 1)], ident[:]
                        )
                        nc.scalar.copy(v_sb[:, kc, :], pv2[:])

                # attention for batch b
                with (
                    tc.tile_pool(name=f"att{b}", bufs=1) as att,
                    tc.tile_pool(name=f"ex{b}", bufs=2) as exp_pool,
                    tc.tile_pool(name=f"au{b}", bufs=5) as aup,
                    tc.tile_pool(name=f"scps{b}", bufs=2, space="PSUM") as scps,
                    tc.tile_pool(name=f"avps{b}", bufs=2, space="PSUM") as avps,
                    tc.tile_pool(name=f"smps{b}", bufs=2, space="PSUM") as smps,
                    tc.tile_pool(name=f"ascps{b}", bufs=2, space="PSUM") as ascps,
                ):
                    sums_b = att.tile([HL, S], F32)
                    a_sums_b = att.tile([HL, S], F32) if not adapter_skip else None
                    attnU = {}
                    a_attnU = {}
                    for h in range(HL):
                        p, beta = h // 2, 64 * (h % 2)
                        sl = slice(beta, beta + 64)
                        rqe, rqo = rqa[p], rqb[p]
                        rke_t = rka if beta == 0 else rkb
                        rko_t = rkb if beta == 0 else rka
                        expT = exp_pool.tile([128, 4, S], F32R, tag="expT")
                        av_p = avps.tile([128, S], F32, tag="av")
                        sm_p = smps.tile([1, S], F32, tag="sm")
                        for kc in range(4):
                            qlo = 128 * kc if causal else 0
                            sc_p = scps.tile([128, S], F32, tag="sc")
                            nc.tensor.matmul(
                                sc_p[:, qlo:S],
                                rke_t[sl, 128 * kc : 128 * (kc + 1)],
                                rqe[sl, qlo:S],
                                start=True, stop=False,
                            )
                            nc.tensor.matmul(
                                sc_p[:, qlo:S],
                                rko_t[sl, 128 * kc : 128 * (kc + 1)],
                                rqo[sl, qlo:S],
                                start=False, stop=True,
                            )
                            nc.scalar.activation(
                                expT[:, kc, qlo:S], sc_p[:, qlo:S],
                                func=mybir.ActivationFunctionType.Exp, scale=SCALE,
                            )
                            if causal:
                                nc.vector.tensor_mul(
                                    expT[:, kc, qlo : qlo + 128],
                                    expT[:, kc, qlo : qlo + 128].bitcast(F32),
                                    em_sb[:],
                                )
                            else:
                                nc.vector.tensor_mul(
                                    expT[:, kc, :],
                                    expT[:, kc, :].bitcast(F32),
                                    em_sb[:, kc, :],
                                )
                            nc.tensor.matmul(
                                sm_p[0:1, qlo:S], ones_r[:, 0:1],
                                expT[:, kc, qlo:S],
                                start=(kc == 0), stop=(kc == 3),
                            )
                            nc.tensor.matmul(
                                av_p[:, qlo:S], v_sb[:, kc, :],
                                expT[:, kc, qlo:S],
                                start=(kc == 0), stop=(kc == 3),
                            )
                        au = aup.tile([128, S], F32, tag="attnU")
                        nc.scalar.copy(au[:], av_p[:])
                        attnU[h] = au
                        smt = aup.tile([1, S], F32, tag="smt")
                        nc.scalar.copy(smt[:], sm_p[0:1, :])
                        nc.sync.dma_start(sums_b[h : h + 1, :], smt[:])

                        if not adapter_skip:
                            asc_p = ascps.tile([64, S], F32, tag="asc")
                            ke_src = akt if beta == 0 else aktb
                            ko_src = aktb if beta == 0 else akt
                            nc.tensor.matmul(
                                asc_p[:], ke_src[sl, b, :], rqe[sl, :],
                                start=True, stop=False,
                            )
                            nc.tensor.matmul(
                                asc_p[:], ko_src[sl, b, :], rqo[sl, :],
                                start=False, stop=True,
                            )
                            a_expT = exp_pool.tile([64, S], F32R, tag="a_expT")
                            nc.scalar.activation(
                                a_expT[:], asc_p[:],
                                func=mybir.ActivationFunctionType.Exp, scale=SCALE,
                            )
                            asm_p = smps.tile([1, S], F32, tag="asm")
                            nc.tensor.matmul(
                                asm_p[0:1, :], ones_r[0:64, 0:1], a_expT[:],
                                start=True, stop=True,
                            )
                            aav_p = avps.tile([128, S], F32, tag="av")
                            nc.tensor.matmul(
                                aav_p[:], av_sb[:, b, :], a_expT[:],
                                start=True, stop=True,
                            )
                            aau = aup.tile([128, S], F32, tag="a_attnU")
                            nc.scalar.copy(aau[:], aav_p[:])
                            a_attnU[h] = aau
                            asmt = aup.tile([1, S], F32, tag="asmt")
                            nc.scalar.copy(asmt[:], asm_p[0:1, :])
                            nc.sync.dma_start(a_sums_b[h : h + 1, :], asmt[:])

                    # reciprocals + normalize + stage for AllGather
                    recip = att.tile([HL, S], F32)
                    nc.vector.reciprocal_approx_fast(recip[:], sums_b[:])
                    recip_d = dram.tile([HL, S], F32, name=f"recip_d{b}")
                    nc.sync.dma_start(recip_d[:], recip[:])
                    if not adapter_skip:
                        a_recip = att.tile([HL, S], F32)
                        nc.vector.reciprocal_approx_fast(a_recip[:], a_sums_b[:])
                        nc.vector.tensor_scalar_mul(a_recip[:], a_recip[:], tg4[:])
                        a_recip_d = dram.tile([HL, S], F32, name=f"a_recip_d{b}")
                        nc.sync.dma_start(a_recip_d[:], a_recip[:])
                    for h in range(HL):
                        rb_bc = aup.tile([128, S], F32, tag="rb_bc")
                        nc.gpsimd.dma_start(
                            rb_bc[:], recip_d[h : h + 1, :].to_broadcast((128, S))
                        )
                        at_n = aup.tile([128, S], F32R, tag="at_n")
                        if adapter_skip:
                            nc.vector.tensor_mul(at_n[:], attnU[h][:], rb_bc[:])
                        else:
                            t_m = aup.tile([128, S], F32, tag="t_m")
                            nc.vector.tensor_mul(t_m[:], attnU[h][:], rb_bc[:])
                            arb_bc = aup.tile([128, S], F32, tag="arb_bc")
                            nc.gpsimd.dma_start(
                                arb_bc[:],
                                a_recip_d[h : h + 1, :].to_broadcast((128, S)),
                            )
                            t_a = aup.tile([128, S], F32, tag="t_a")
                            nc.vector.tensor_mul(t_a[:], a_attnU[h][:], arb_bc[:])
                            nc.vector.tensor_add(at_n[:], t_m[:], t_a[:])
                        nc.sync.dma_start(
                            at_in[b][128 * h : 128 * (h + 1), :], at_n[:]
                        )
                nc.gpsimd.collective_compute(
                    "AllGather", mybir.AluOpType.bypass, replica_groups=rg,
                    ins=[at_in[b][:]], outs=[at_full[b][:]],
                )

            # ---- wo (column shard), per 2-batch chunk ----
            for c in range(2):
                with (
                    tc.tile_pool(name=f"wo{c}", bufs=3) as wop,
                    tc.tile_pool(name=f"woo{c}", bufs=2) as woo,
                    tc.tile_pool(name=f"wops{c}", bufs=1, space="PSUM") as wops,
                ):
                    o_ps = [
                        wops.tile([128, 2, 512], F32, name=f"ops{m}") for m in range(4)
                    ]
                    for kt in range(32):
                        rhs_t = wop.tile([128, 2, 512], F32R, tag="rhs")
                        for sub in range(2):
                            nc.sync.dma_start(
                                rhs_t[:, sub, :],
                                at_full[2 * c + sub][128 * kt : 128 * (kt + 1), :],
                            )
                        slab = wop.tile([128, 512], F32R, tag="slab")
                        nc.sync.dma_start(
                            slab[:], wo_r[128 * kt : 128 * (kt + 1), :]
                        )
                        for m in range(4):
                            for sub in range(2):
                                nc.tensor.matmul(
                                    o_ps[m][:, sub, :],
                                    slab[:, 128 * m : 128 * (m + 1)],
                                    rhs_t[:, sub, :],
                                    start=(kt == 0), stop=(kt == 31),
                                )
                    for m in range(4):
                        osb = woo.tile([128, 2, 512], F32, tag="osb")
                        nc.scalar.copy(osb[:], o_ps[m][:])
                        nc.sync.dma_start(
                            out_r[
                                128 * m : 128 * (m + 1),
                                1024 * c : 1024 * (c + 1),
                            ].rearrange("p (s t) -> p s t", s=2),
                            osb[:],
                        )

    nc.compile()
    return nc


def kernel(**inputs) -> np.ndarray:
    in_maps, causal, adapter_skip = _host_prep(inputs)
    key = (causal, adapter_skip)
    if key not in _cache:
        _cache[key] = _build(causal, adapter_skip)
    nc = _cache[key]
    res = run_bass_kernel_spmd(nc, in_maps, core_ids=list(range(NCORES)))
    global last_result
    last_result = res
    out = np.empty((B * S, D), np.float32)
    for r in range(NCORES):
        out[:, 512 * r : 512 * (r + 1)] = res.results[r]["out_r"].T
    return out.reshape(B, S, D)


if __name__ == "__main__":
    rng = np.random.default_rng(0)
    demo = {
        "x": rng.standard_normal((B, S, D), dtype=np.float32),
        "adapter": rng.standard_normal((B, A_LEN, D), dtype=np.float32),
        "mask": np.where(
            np.tril(np.ones((S, S), dtype=bool)), 0.0, -1e9
        ).astype(np.float32)[None, None],
        "freqs_cos": rng.random((S, 64), dtype=np.float32),
        "freqs_sin": rng.random((S, 64), dtype=np.float32),
        "wq": (rng.standard_normal((D, H * HD), dtype=np.float32) * 0.02),
        "wk": (rng.standard_normal((D, HK * HD), dtype=np.float32) * 0.02),
        "wv": (rng.standard_normal((D, HK * HD), dtype=np.float32) * 0.02),
        "wo": (rng.standard_normal((H * HD, D), dtype=np.float32) * 0.02),
        "gate": np.zeros((1, H, 1, 1), np.float32),
    }
    o = kernel(**demo)
    print("kernel ran, out shape", o.shape)
